# revision 2
# baseline (speedup 1.0000x reference)
"""Ensemble-KF unimodal-fusion forward, data-parallel across 8 NeuronCores.

Contract: kernel(**inputs) takes FULL unsharded numpy inputs, shards the
batch (256 -> 8 x 32) across the 8 cores with params replicated, runs the
SPMD program, and returns the FULL output tuple (same structure as the
reference _ekf_forward). Self-contained: hardcodes all shapes.

The graph is written transpose-free (einsum batch contractions) and the
16x16 inverses use an unrolled Gauss-Jordan with the pivot-row restore
folded into the rank-1 update -- both to stay on lowering paths the
Neuron compiler handles well, and because the matrices are PD so no
pivoting is needed.
"""

import os

os.environ.setdefault("NEURON_CC_FLAGS", "--auto-cast=none")

import numpy as np
import jax
import jax.numpy as jnp

EN, DX, DZ = 64, 32, 16
B = 256
NDEV = 8
BS = B // NDEV
R_DIAG, CONST = 0.1, 0.001


def _mlp(x, layers):
    for i, (W, b) in enumerate(layers):
        x = x @ W + b
        if i < len(layers) - 1:
            x = jax.nn.relu(x)
    return x


def _inv(A):
    # Batched Gauss-Jordan inverse without pivoting (inputs are PD).
    # A: [n, d, d]. Row k is restored by subtracting (col - e_k) x piv,
    # avoiding dynamic-update-slice ops.
    n, d = A.shape[0], A.shape[-1]
    eye = jnp.broadcast_to(jnp.eye(d, dtype=A.dtype), A.shape)
    M = jnp.concatenate([A, eye], axis=-1)  # [n, d, 2d]
    ident = np.eye(d, dtype=np.float32)
    for k in range(d):
        piv = M[:, k, :] / M[:, k, k : k + 1]  # [n, 2d]
        col = M[:, :, k] - ident[k][None, :]   # [n, d]
        M = M - col[:, :, None] * piv[:, None, :]
    return M[:, :, d:]


def _forward(obs_img, obs_1, state_old, init_ensemble, params):
    bs = obs_img.shape[0]
    init_e = jnp.broadcast_to(init_ensemble[None], (bs, EN, DZ))

    flat = state_old.reshape(bs * EN, DX)
    state_pred = (flat + _mlp(flat, params['proc'])).reshape(bs, EN, DX)

    m_A = state_pred.mean(axis=1)                  # [bs, dx]
    Ac = state_pred - m_A[:, None, :]              # [bs, en, dx]

    H_X = _mlp(state_pred.reshape(bs * EN, DX), params['obs']).reshape(bs, EN, DZ)
    mean = H_X.mean(axis=1)
    H_X_mean = mean[:, None, :]
    H_A = H_X - mean[:, None, :]                   # [bs, en, dz]

    ens1 = _mlp(obs_img.reshape(bs, -1), params['img_enc']).reshape(bs, EN, DZ)
    ens2 = _mlp(obs_1.reshape(bs, -1), params['sens_enc']).reshape(bs, EN, DZ)
    m1 = ens1.mean(axis=1)[:, None, :]
    m2 = ens2.mean(axis=1)[:, None, :]

    c1 = ens1 - m1                                 # [bs, en, dz]
    c2 = ens2 - m2
    cov1 = jnp.einsum('bei,bej->bij', c1, c1) / (EN - 1)
    cov2 = jnp.einsum('bei,bej->bij', c2, c2) / (EN - 1)
    cov_fuse_ = _inv(_inv(cov1) + _inv(cov2))
    ensemble_z = jnp.einsum('bez,bzw->bew', init_e, cov_fuse_)  # [bs, en, dz]
    z = ensemble_z.mean(axis=1)[:, None, :]

    init_c = np.sqrt(R_DIAG * R_DIAG - CONST).astype(np.float32)
    diag = _mlp(z[:, 0, :], params['noise'])
    diag = jnp.square(diag + CONST) + init_c
    R = diag[:, :, None] * jnp.eye(DZ, dtype=diag.dtype)

    innovation = jnp.einsum('bei,bej->bij', H_A, H_A) / (EN - 1) + R
    M_ = jnp.einsum('bed,bez->bdz', Ac, H_A) / (EN - 1)   # [bs, dx, dz]
    K = jnp.einsum('bdz,bzw->bdw', M_, _inv(innovation))  # [bs, dx, dz]
    gain = jnp.einsum('bdz,bez->bed', K, ensemble_z - H_X)  # [bs, en, dx]
    state_new = state_pred + gain

    m_state_new = state_new.mean(axis=1)[:, None, :]
    m_state_pred = m_A[:, None, :]
    return (state_new, m_state_new, m_state_pred, m1, m2, z, ensemble_z, H_X_mean)


_fwd_pmap = jax.pmap(_forward, in_axes=(0, 0, 0, None, None))


def _shard(x):
    x = np.ascontiguousarray(np.asarray(x))
    return x.reshape((NDEV, BS) + x.shape[1:])


def _unshard(y):
    y = np.asarray(y)
    return y.reshape((y.shape[0] * y.shape[1],) + y.shape[2:])


def run_sharded(obs_img, obs_1, state_old, init_ensemble, params):
    """Dispatch the SPMD program across the 8 cores; returns device outputs."""
    return _fwd_pmap(
        _shard(obs_img), _shard(obs_1), _shard(state_old),
        jnp.asarray(np.asarray(init_ensemble)),
        jax.tree.map(lambda a: jnp.asarray(np.asarray(a)), params),
    )


def kernel(obs_img, obs_1, state_old, m_state, init_ensemble, params):
    outs = run_sharded(obs_img, obs_1, state_old, init_ensemble, params)
    return tuple(_unshard(o) for o in outs)
